# revision 12
# baseline (speedup 1.0000x reference)
"""BBox-aware BCE loss kernel for Trainium2 (8 NeuronCores, data parallel).

Math (exact reformulation of the reference):
  loss = softplus(pred) - pred*target = softplus((1-2t)*pred)   for t in {0,1}
  u = 1-2t in {+1 (t=0), -1 (t=1)}
  Su(i,j) = sum of u over the clipped 5x5 window = (#zeros - #ones)
  edge pixel  <=>  window is mixed  <=>  |Su| < V   (V = clipped window size)
  weights w = 1 - 0.9*edge  (and the reference's global `cond` branch is a
  no-op: if no edge exists anywhere, sum(loss*edge)=0, so
  mean(loss*w) = (sum(loss) - 0.9*sum(loss*edge))/N in both branches).

Per-core device pipeline (4 samples/core, 9 overlapping 128-row tiles per
sample so the 5-tap column window never crosses a tile boundary):
  ScalarE: u-encode (Copy, scale=-2,bias=1 -> bf16), Softplus(+accum sum),
           PSUM->SBUF copy of column sums
  TensorE: 5-tap column window sum via banded-matrix matmul (bf16)
  VectorE: s = p_bf*u, 2 of 3 row-window adds, |Su| (abs_max), edge-V
           adjustments, fused (|Su|<24.5)*loss with accumulated sum
  GpSimd:  pred f32->bf16 cast, 1 row-window add
Outputs [128,2] f32 per core (per-partition sums of loss and loss*edge);
host reduces in float64.
"""

import os
import sys

import numpy as np

sys.path.insert(0, "/opt/trn_rl_repo")

import ml_dtypes

B, H, W = 32, 1024, 1024
NCORES = 8
SPC = B // NCORES  # samples per core
ROWS = SPC * H  # dram rows per core
N_TOT = float(B * H * W)

# per-sample tiling: (input_row_start, input_rows, owned_lo, owned_hi)
TILES = []
TILES.append((0, 128, 0, 126))
for t in range(1, 8):
    TILES.append((124 * t, 128, 2, 126))
TILES.append((992, 32, 2, 32))
NT = len(TILES)  # 9
NTILES = SPC * NT  # 36

BF16 = ml_dtypes.bfloat16


def _band(k_rows: int, m_lo: int, m_hi: int) -> np.ndarray:
    a = np.zeros((k_rows, 128), dtype=np.float32)
    for k in range(k_rows):
        for m in range(m_lo, m_hi):
            if abs(k - m) <= 2:
                a[k, m] = 1.0
    return a.astype(BF16)


def _vh_of(tile_idx: int) -> np.ndarray:
    """clipped column-window size per in-tile row (only owned rows matter)."""
    in0, p_in, _, _ = TILES[tile_idx]
    vh = np.full(128, 5.0, dtype=np.float32)
    for k in range(p_in):
        img = in0 + k
        vh[k] = min(img, 2) + min(H - 1 - img, 2) + 1
    return vh


def _statics() -> dict[str, np.ndarray]:
    vw_edge = np.array([3.0, 4.0, 4.0, 3.0], dtype=np.float32)  # cols 0,1,W-2,W-1
    s = {
        "a_top": _band(128, 0, 126),
        "a_mid": _band(128, 2, 126),
        "a_last": _band(32, 2, 32)[:32],
        # adj[k,c] = 25 - vh(k)*vw_edge(c): added to |Su| on the 4 edge cols
        "adj_top": (25.0 - np.outer(_vh_of(0), vw_edge)).astype(BF16),
        "adj_mid": (25.0 - np.outer(_vh_of(1), vw_edge)).astype(BF16),
        "adj_last": (25.0 - np.outer(_vh_of(8), vw_edge)).astype(BF16),
        # row-strip adders (interior cols): 25 - vh*5 per in-tile row.
        # col0 = top tile (rows 0,1 -> 10,5), col1 = last tile (rows 30,31)
        "adjrow": _adjrow(),
        # owned-row masks (1=owned) and non-owned bias (+100) per tile type
        "ones_top": _owned(0).astype(BF16).reshape(128, 1),
        "ones_mid": _owned(1).astype(BF16).reshape(128, 1),
        "ones_last": _owned(8).astype(BF16).reshape(128, 1),
        "bias_top": (100.0 * (1.0 - _owned(0))).astype(np.float32).reshape(128, 1),
        "bias_mid": (100.0 * (1.0 - _owned(1))).astype(np.float32).reshape(128, 1),
        "bias_last": (100.0 * (1.0 - _owned(8))).astype(np.float32).reshape(128, 1),
    }
    return s


def _owned(tile_idx: int) -> np.ndarray:
    _, _, o0, o1 = TILES[tile_idx]
    m = np.zeros(128, dtype=np.float32)
    m[o0:o1] = 1.0
    return m


def _adjrow() -> np.ndarray:
    a = np.zeros((128, 2), dtype=np.float32)
    a[0, 0], a[1, 0] = 10.0, 5.0   # top tile img rows 0,1
    a[30, 1], a[31, 1] = 5.0, 10.0  # last tile img rows 1022,1023
    return a


_CACHED = {}


def _build_nc():
    import concourse.bass as bass
    import concourse.mybir as mybir
    import concourse.tile as tile

    f32 = mybir.dt.float32
    bf16 = mybir.dt.bfloat16
    Act = mybir.ActivationFunctionType
    Alu = mybir.AluOpType

    nc = bass.Bass("TRN2", target_bir_lowering=False, debug=False,
                   num_devices=NCORES)

    pred_d = nc.dram_tensor("pred", [ROWS, W], f32, kind="ExternalInput").ap()
    tgt_d = nc.dram_tensor("target", [ROWS, W], f32, kind="ExternalInput").ap()
    a_top_d = nc.dram_tensor("a_top", [128, 128], bf16, kind="ExternalInput").ap()
    a_mid_d = nc.dram_tensor("a_mid", [128, 128], bf16, kind="ExternalInput").ap()
    a_last_d = nc.dram_tensor("a_last", [32, 128], bf16, kind="ExternalInput").ap()
    adj_top_d = nc.dram_tensor("adj_top", [128, 4], bf16, kind="ExternalInput").ap()
    adj_mid_d = nc.dram_tensor("adj_mid", [128, 4], bf16, kind="ExternalInput").ap()
    adj_last_d = nc.dram_tensor("adj_last", [128, 4], bf16, kind="ExternalInput").ap()
    adjrow_d = nc.dram_tensor("adjrow", [128, 2], f32, kind="ExternalInput").ap()
    ones_top_d = nc.dram_tensor("ones_top", [128, 1], bf16, kind="ExternalInput").ap()
    ones_mid_d = nc.dram_tensor("ones_mid", [128, 1], bf16, kind="ExternalInput").ap()
    ones_last_d = nc.dram_tensor("ones_last", [128, 1], bf16, kind="ExternalInput").ap()
    bias_top_d = nc.dram_tensor("bias_top", [128, 1], f32, kind="ExternalInput").ap()
    bias_mid_d = nc.dram_tensor("bias_mid", [128, 1], f32, kind="ExternalInput").ap()
    bias_last_d = nc.dram_tensor("bias_last", [128, 1], f32, kind="ExternalInput").ap()
    out_d = nc.dram_tensor("out", [128, 2], f32, kind="ExternalOutput").ap()
    out2_d = nc.dram_tensor("out2", [1, 1024], f32, kind="ExternalOutput").ap()

    WP = W + 4  # padded width for the 5-tap row window

    with tile.TileContext(nc) as tc:
        with (
            tc.tile_pool(name="sing", bufs=1) as sing,
            tc.tile_pool(name="tgt", bufs=3) as tgt_pool,
            tc.tile_pool(name="prd", bufs=3) as prd_pool,
            tc.tile_pool(name="pb", bufs=3) as pb_pool,
            tc.tile_pool(name="loss", bufs=3) as loss_pool,
            tc.tile_pool(name="g", bufs=2) as g_pool,
            tc.tile_pool(name="w1", bufs=2) as w1_pool,
            tc.tile_pool(name="w2", bufs=2) as w2_pool,
            tc.tile_pool(name="su", bufs=2) as su_pool,
            tc.tile_pool(name="asu", bufs=2) as asu_pool,
            tc.tile_pool(name="scr", bufs=2) as scr_pool,
            tc.tile_pool(name="psum", bufs=2, space="PSUM") as psum_pool,
        ):
            # ---- statics in SBUF ----
            a_top = sing.tile([128, 128], bf16, tag="a_top")
            a_mid = sing.tile([128, 128], bf16, tag="a_mid")
            a_last = sing.tile([32, 128], bf16, tag="a_last")
            adj_top = sing.tile([128, 4], bf16, tag="adj_top")
            adj_mid = sing.tile([128, 4], bf16, tag="adj_mid")
            adj_last = sing.tile([128, 4], bf16, tag="adj_last")
            adjrow = sing.tile([128, 2], f32, tag="adjrow")
            ones_top = sing.tile([128, 1], bf16, tag="ones_top")
            ones_mid = sing.tile([128, 1], bf16, tag="ones_mid")
            ones_last = sing.tile([128, 1], bf16, tag="ones_last")
            bias_top = sing.tile([128, 1], f32, tag="bias_top")
            bias_mid = sing.tile([128, 1], f32, tag="bias_mid")
            bias_last = sing.tile([128, 1], f32, tag="bias_last")
            for sb, dr in ((a_top, a_top_d), (a_mid, a_mid_d), (a_last, a_last_d),
                           (adj_top, adj_top_d), (adj_mid, adj_mid_d),
                           (adj_last, adj_last_d), (adjrow, adjrow_d),
                           (ones_top, ones_top_d), (ones_mid, ones_mid_d),
                           (ones_last, ones_last_d), (bias_top, bias_top_d),
                           (bias_mid, bias_mid_d), (bias_last, bias_last_d)):
                nc.sync.dma_start(out=sb[:], in_=dr[:])
            a_by = {0: a_top, 8: a_last}
            adj_by = {0: adj_top, 8: adj_last}
            ones_by = {0: ones_top, 8: ones_last}
            bias_by = {0: bias_top, 8: bias_last}

            # whole-kernel PSUM accumulator for sum(loss) (two 512-col banks)
            lsum = psum_pool.tile([1, 1024], f32, tag="lsum")

            # stats: cols [0,NTILES) = per-tile loss sums, [40,40+NTILES) = le
            stats = sing.tile([128, 80], f32, tag="stats")
            nc.vector.memset(stats[:], 0.0)

            # padded ring buffers (pads zeroed once, never rewritten)
            u_bufs = [sing.tile([128, WP], bf16, tag=f"ub{i}", name=f"ub{i}") for i in range(3)]
            cu_bufs = [sing.tile([128, WP], bf16, tag=f"cb{i}", name=f"cb{i}") for i in range(3)]
            for bb in u_bufs + cu_bufs:
                nc.vector.memset(bb[:, 0:2], 0.0)
                nc.vector.memset(bb[:, W + 2:W + 4], 0.0)

            idx = 0
            for smp in range(SPC):
                for t in range(NT):
                    in0, p_in, o0, o1 = TILES[t]
                    r0 = smp * H + in0
                    a_sb = a_by.get(t, a_mid)
                    adj_sb = adj_by.get(t, adj_mid)
                    ones_sb = ones_by.get(t, ones_mid)
                    bias_sb = bias_by.get(t, bias_mid)

                    tgt = tgt_pool.tile([128, W], f32)
                    nc.sync.dma_start(out=tgt[0:p_in], in_=tgt_d[r0:r0 + p_in, :])
                    prd = prd_pool.tile([128, W], f32)
                    nc.sync.dma_start(out=prd[0:p_in], in_=pred_d[r0:r0 + p_in, :])

                    # u = 1 - 2t (bf16), into padded buffer center
                    ub = u_bufs[idx % 3]
                    nc.scalar.activation(out=ub[0:p_in, 2:2 + W], in_=tgt[0:p_in],
                                         func=Act.Copy, bias=1.0, scale=-2.0)

                    # s = bf16(pred) * u  (in place over the cast)
                    pb = pb_pool.tile([128, W], bf16)
                    nc.gpsimd.tensor_copy(out=pb[0:p_in], in_=prd[0:p_in])
                    nc.vector.tensor_mul(out=pb[0:p_in], in0=pb[0:p_in],
                                         in1=ub[0:p_in, 2:2 + W])

                    # loss = softplus(s) = ln(1+exp(s)); Softplus has no ACT
                    # table set, but exp+ln co-reside in one. s in [-6,6] so
                    # exp stays in range.
                    g = g_pool.tile([128, W], bf16)
                    nc.scalar.activation(out=g[0:p_in], in_=pb[0:p_in],
                                         func=Act.Exp)
                    loss = loss_pool.tile([128, W], bf16)
                    nc.scalar.activation(out=loss[0:p_in], in_=g[0:p_in],
                                         func=Act.Ln, bias=1.0)
                    for h in (0, 512):
                        nc.tensor.matmul(lsum[:, h:h + 512],
                                         ones_sb[0:p_in, :],
                                         loss[0:p_in, h:h + 512],
                                         start=(idx == 0), stop=(idx == NTILES - 1))

                    # column 5-window sum via banded matmul (PSUM f32)
                    cup = psum_pool.tile([128, W], f32)
                    for h in (0, 512):
                        nc.tensor.matmul(cup[:, h:h + 512], a_sb[0:p_in, :],
                                         ub[0:p_in, 2 + h:2 + h + 512],
                                         start=True, stop=True)
                    # PSUM->SBUF with per-row bias: +100 on non-owned rows
                    # forces |Su| >> 24.5 there, so those rows contribute 0
                    # to the edge-weighted sum (no overlap double-count).
                    cub = cu_bufs[idx % 3]
                    nc.scalar.activation(out=cub[:, 2:2 + W], in_=cup[:],
                                         func=Act.Identity, bias=bias_sb[:])

                    # row 5-window sum: Su = sum_{d=-2..2} Cu[:, j+d]
                    w1 = w1_pool.tile([128, WP], bf16)
                    nc.vector.tensor_add(out=w1[:, 0:WP - 1], in0=cub[:, 0:WP - 1],
                                         in1=cub[:, 1:WP])
                    w2 = w2_pool.tile([128, WP], bf16)
                    nc.gpsimd.tensor_add(out=w2[:, 0:WP - 3], in0=w1[:, 0:WP - 3],
                                         in1=w1[:, 2:WP - 1])
                    su = su_pool.tile([128, W], bf16)
                    nc.vector.tensor_add(out=su[:], in0=w2[:, 0:W],
                                         in1=cub[:, 4:WP])

                    # |Su|, then add (25-V) on image-edge strips
                    asu = asu_pool.tile([128, W], bf16)
                    nc.vector.scalar_tensor_tensor(
                        out=asu[:], in0=su[:], scalar=-1.0, in1=su[:],
                        op0=Alu.mult, op1=Alu.max)
                    asu_edges = bass.AP(
                        tensor=asu[:].tensor, offset=asu[:].offset,
                        ap=[asu[:].ap[0], [W - 2, 2], [1, 2]])
                    nc.vector.tensor_add(
                        out=asu_edges, in0=asu_edges,
                        in1=adj_sb[:].rearrange("p (a b) -> p a b", b=2))
                    if t == 0:
                        nc.vector.tensor_scalar(
                            out=asu[0:2, 2:W - 2], in0=asu[0:2, 2:W - 2],
                            scalar1=adjrow[0:2, 0:1], scalar2=None, op0=Alu.add)
                    elif t == NT - 1:
                        nc.vector.tensor_scalar(
                            out=asu[0:32, 2:W - 2], in0=asu[0:32, 2:W - 2],
                            scalar1=adjrow[0:32, 1:2], scalar2=None, op0=Alu.add)

                    # le = (|Su|' < 24.5) * loss, accumulated per partition
                    scr = scr_pool.tile([128, W], bf16)
                    nc.vector.scalar_tensor_tensor(
                        out=scr[0:p_in], in0=asu[0:p_in], scalar=24.5,
                        in1=loss[0:p_in], op0=Alu.is_lt, op1=Alu.mult,
                        accum_out=stats[0:p_in, 40 + idx:41 + idx])
                    idx += 1

            red = sing.tile([128, 2], f32, tag="red")
            nc.vector.memset(red[:, 0:1], 0.0)
            nc.vector.reduce_sum(out=red[:, 1:2], in_=stats[:, 40:40 + NTILES],
                                 axis=mybir.AxisListType.X)
            nc.sync.dma_start(out=out_d[:], in_=red[:])
            lsum_sb = sing.tile([1, 1024], f32, tag="lsum_sb")
            nc.vector.tensor_copy(out=lsum_sb[:], in_=lsum[:])
            nc.sync.dma_start(out=out2_d[:], in_=lsum_sb[:])

    _split_multi_waits(nc, mybir)
    return nc


def _split_multi_waits(nc, mybir):
    """This walrus's core_v3 codegen allows only one sem-wait per compute
    instruction; peel extra waits onto same-engine NOPs placed just before."""
    skip = (mybir.InstEventSemaphore,)
    k = 0
    for fn in nc.m.functions:
        for blk in fn.blocks:
            out = []
            for inst in blk.instructions:
                si = inst.sync_info
                if (si is not None and len(si.on_wait) > 1
                        and not isinstance(inst, skip)):
                    waits = list(si.on_wait)
                    for w in waits[:-1]:
                        k += 1
                        nop = mybir.InstNoOp(name=f"wsplit-{k}", ins=[], outs=[])
                        nop.engine = inst.engine
                        nop.sync_info = mybir.SyncInfo(on_wait=[w], on_update=[])
                        out.append(nop)
                    inst.sync_info = mybir.SyncInfo(
                        on_wait=[waits[-1]], on_update=list(si.on_update))
                out.append(inst)
            blk.instructions = out


def _get_nc():
    if "nc" not in _CACHED:
        _CACHED["nc"] = _build_nc()
    return _CACHED["nc"]


def run(pred: np.ndarray, target: np.ndarray, trace: bool = False):
    """Returns (result_scalar, BassKernelResults)."""
    from concourse import bass_utils

    nc = _get_nc()
    statics = _statics()
    pred = np.ascontiguousarray(np.asarray(pred).reshape(B * H, W), dtype=np.float32)
    target = np.ascontiguousarray(np.asarray(target).reshape(B * H, W), dtype=np.float32)
    in_maps = []
    for c in range(NCORES):
        m = dict(statics)
        m["pred"] = pred[c * ROWS:(c + 1) * ROWS]
        m["target"] = target[c * ROWS:(c + 1) * ROWS]
        in_maps.append(m)
    res = bass_utils.run_bass_kernel_spmd(
        nc, in_maps, core_ids=list(range(NCORES)), trace=trace)
    s_loss = 0.0
    s_le = 0.0
    for r in res.results:
        s_loss += r["out2"].astype(np.float64).sum()
        s_le += r["out"].astype(np.float64)[:, 1].sum()
    val = np.float32((s_loss - 0.9 * s_le) / N_TOT)
    return np.asarray(val, dtype=np.float32), res


def kernel(pred: np.ndarray, target: np.ndarray) -> np.ndarray:
    val, _ = run(pred, target, trace=False)
    return val


if __name__ == "__main__":
    rng = np.random.default_rng(0)
    p = rng.standard_normal((B, 1, H, W), dtype=np.float32)
    t = rng.integers(0, 2, (B, 1, H, W)).astype(np.float32)
    print(kernel(pred=p, target=t))


# revision 15
# speedup vs baseline: 1.8308x; 1.8308x over previous
"""BBox-aware BCE loss kernel for Trainium2 (8 NeuronCores, data parallel).

Math (exact reformulation of the reference):
  loss = softplus(pred) - pred*target = softplus((1-2t)*pred)   for t in {0,1}
  u = 1-2t in {+1 (t=0), -1 (t=1)}
  Su(i,j) = sum of u over the clipped 5x5 window = (#zeros - #ones)
  edge pixel  <=>  window is mixed  <=>  |Su| < V   (V = clipped window size)
  result = (sum(loss) - 0.9*sum(loss*edge)) / N   (equals the reference in
  both branches of its global `cond`: no edges anywhere => sum(loss*edge)=0).

Device pipeline per core (4 samples, 9 overlapping 128-row tiles each so the
5-tap column window never crosses a tile boundary; owned rows exclude the
2-row overlap):
  GpSimd: casting DMAs (f32 HBM -> bf16 SBUF) for pred/target; first
          row-window pair-add v[j] = u[j]+u[j+1]
  VectorE: u = 1-2t; s = p*u; fused (|Su|<V)*loss with accumulated sums
           (main + image-edge strips with their own thresholds)
  ScalarE: g = exp(s); loss = ln(g*own + 1) (own=0 kills overlap rows);
           |Su| via Abs on the PSUM->SBUF copy
  TensorE: column 5-window sum via 3 shifted accumulating band matmuls
           (Su = A@(v(-2) + v(0) + u(+2))); sum(loss) via ones-matmuls
           accumulated in PSUM over all tiles
Host: float64 reduction of per-core partials.
"""

import sys

import numpy as np

sys.path.insert(0, "/opt/trn_rl_repo")

import ml_dtypes

B, H, W = 32, 1024, 1024
NCORES = 8
SPC = B // NCORES  # samples per core
ROWS = SPC * H
N_TOT = float(B * H * W)

# per-sample tiling: (input_row_start, input_rows, owned_lo, owned_hi)
TILES = [(0, 128, 0, 126)]
for t in range(1, 8):
    TILES.append((124 * t, 128, 2, 126))
TILES.append((992, 32, 2, 32))
NT = len(TILES)  # 9
NTILES = SPC * NT  # 36

BF16 = ml_dtypes.bfloat16


def _band(k_rows: int, m_lo: int, m_hi: int) -> np.ndarray:
    a = np.zeros((k_rows, 128), dtype=np.float32)
    for k in range(k_rows):
        for m in range(m_lo, m_hi):
            if abs(k - m) <= 2:
                a[k, m] = 1.0
    return a.astype(BF16)


def _vh_of(tile_idx: int) -> np.ndarray:
    """clipped column-window size per in-tile row."""
    in0, p_in, _, _ = TILES[tile_idx]
    vh = np.full(128, 5.0, dtype=np.float32)
    for k in range(p_in):
        img = in0 + k
        vh[k] = min(img, 2) + min(H - 1 - img, 2) + 1
    return vh


def _owned(tile_idx: int) -> np.ndarray:
    _, _, o0, o1 = TILES[tile_idx]
    m = np.zeros(128, dtype=np.float32)
    m[o0:o1] = 1.0
    return m


def _statics() -> dict[str, np.ndarray]:
    s = {}
    s["a_top"] = _band(128, 0, 126)
    s["a_mid"] = _band(128, 2, 126)
    s["a_last"] = _band(32, 2, 32)
    for nm, ti in (("top", 0), ("mid", 1), ("last", 8)):
        vh = _vh_of(ti)
        own = _owned(ti)
        edge_row = (vh < 5.0) & (own > 0)  # image top/bottom rows (owned)
        s[f"ones_{nm}"] = own.astype(BF16).reshape(128, 1)
        # Per-partition |Su| thresholds; -1 disables a row (never an edge,
        # contributes 0) so the four accumulation regions are exactly
        # disjoint and overlap rows are excluded everywhere.
        # main: interior cols, vh=5 owned rows only
        thrm = np.where(own > 0, 24.5, -1.0)
        thrm = np.where(edge_row, -1.0, thrm)
        s[f"thrm_{nm}"] = thrm.reshape(128, 1).astype(np.float32)
        # edge cols {0,W-1}: vw=3; {1,W-2}: vw=4 (all owned rows)
        s[f"thra_{nm}"] = np.where(own > 0, vh * 3.0 - 0.5, -1.0).reshape(
            128, 1).astype(np.float32)
        s[f"thrb_{nm}"] = np.where(own > 0, vh * 4.0 - 0.5, -1.0).reshape(
            128, 1).astype(np.float32)
        # image top/bottom rows, interior cols (vw=5)
        s[f"thrr_{nm}"] = np.where(edge_row, vh * 5.0 - 0.5, -1.0).reshape(
            128, 1).astype(np.float32)
    return s


_CACHED = {}


def _split_multi_waits(nc, mybir):
    """This walrus's core_v3 codegen allows only one sem-wait per
    instruction; peel extra waits onto same-engine NOPs placed just before."""
    skip = (mybir.InstEventSemaphore,)
    k = 0
    for fn in nc.m.functions:
        for blk in fn.blocks:
            out = []
            for inst in blk.instructions:
                si = inst.sync_info
                if (si is not None and len(si.on_wait) > 1
                        and not isinstance(inst, skip)):
                    waits = list(si.on_wait)
                    for w in waits[:-1]:
                        k += 1
                        nop = mybir.InstNoOp(name=f"wsplit-{k}", ins=[], outs=[])
                        nop.engine = inst.engine
                        nop.sync_info = mybir.SyncInfo(on_wait=[w], on_update=[])
                        out.append(nop)
                    inst.sync_info = mybir.SyncInfo(
                        on_wait=[waits[-1]], on_update=list(si.on_update))
                out.append(inst)
            blk.instructions = out


def _build_nc():
    import concourse.bass as bass
    import concourse.mybir as mybir
    import concourse.tile as tile

    f32 = mybir.dt.float32
    bf16 = mybir.dt.bfloat16
    Act = mybir.ActivationFunctionType
    Alu = mybir.AluOpType

    nc = bass.Bass("TRN2", target_bir_lowering=False, debug=False,
                   num_devices=NCORES)

    pred_d = nc.dram_tensor("pred", [ROWS, W], f32, kind="ExternalInput").ap()
    tgt_d = nc.dram_tensor("target", [ROWS, W], f32, kind="ExternalInput").ap()
    sd = {}
    statics = _statics()
    for nm, arr in statics.items():
        dt = bf16 if arr.dtype == BF16 else f32
        sd[nm] = nc.dram_tensor(nm, list(arr.shape), dt,
                                kind="ExternalInput").ap()
    out_d = nc.dram_tensor("out", [128, 2], f32, kind="ExternalOutput").ap()
    out2_d = nc.dram_tensor("out2", [1, 1024], f32, kind="ExternalOutput").ap()

    WP = W + 4  # padded width for the 5-tap row window

    with tile.TileContext(nc) as tc:
        with (
            tc.tile_pool(name="sing", bufs=1) as sing,
            tc.tile_pool(name="tb", bufs=3) as tb_pool,
            tc.tile_pool(name="pb", bufs=3) as pb_pool,
            tc.tile_pool(name="g", bufs=2) as g_pool,
            tc.tile_pool(name="loss", bufs=3) as loss_pool,
            tc.tile_pool(name="asu", bufs=2) as asu_pool,
            tc.tile_pool(name="scr", bufs=2) as scr_pool,
            tc.tile_pool(name="psum", bufs=2, space="PSUM") as psum_pool,
            tc.tile_pool(name="psum1", bufs=1, space="PSUM") as psum1_pool,
        ):
            # ---- statics in SBUF ----
            sb = {}
            for nm, arr in statics.items():
                dt = bf16 if arr.dtype == BF16 else f32
                sb[nm] = sing.tile(list(arr.shape), dt, tag=nm, name=nm)
                nc.sync.dma_start(out=sb[nm][:], in_=sd[nm][:])

            def per_tile(t):
                nm = "top" if t == 0 else ("last" if t == NT - 1 else "mid")
                return (sb[f"a_{nm}"], sb[f"ones_{nm}"], sb[f"thrm_{nm}"],
                        sb[f"thra_{nm}"], sb[f"thrb_{nm}"], sb[f"thrr_{nm}"])

            # stats columns: [0:36) main le, [40:76) colA, [80:116) colB,
            # [120:128) row strips
            stats = sing.tile([128, 128], f32, tag="stats")
            nc.vector.memset(stats[:], 0.0)

            # whole-kernel PSUM accumulator for sum(loss)
            lsum = psum1_pool.tile([1, 1024], f32, tag="lsum")

            # padded ring buffers (pads zeroed once, never rewritten)
            u_bufs = [sing.tile([128, WP], bf16, tag=f"ub{i}", name=f"ub{i}")
                      for i in range(3)]
            v_bufs = [sing.tile([128, WP], bf16, tag=f"vb{i}", name=f"vb{i}")
                      for i in range(3)]
            for bb in u_bufs + v_bufs:
                nc.vector.memset(bb[:, 0:2], 0.0)
                nc.vector.memset(bb[:, W + 2:WP], 0.0)

            idx = 0
            rowidx = 0
            for smp in range(SPC):
                for t in range(NT):
                    in0, p_in, o0, o1 = TILES[t]
                    r0 = smp * H + in0
                    a_sb, ones_sb, thrm_sb, thra_sb, thrb_sb, thrr_sb = per_tile(t)

                    # casting DMAs: f32 HBM -> bf16 SBUF
                    tb = tb_pool.tile([128, W], bf16)
                    nc.gpsimd.dma_start(out=tb[0:p_in],
                                        in_=tgt_d[r0:r0 + p_in, :])
                    pb = pb_pool.tile([128, W], bf16)
                    nc.gpsimd.dma_start(out=pb[0:p_in],
                                        in_=pred_d[r0:r0 + p_in, :])

                    # u = 1 - 2t into padded buffer center
                    ub = u_bufs[idx % 3]
                    nc.vector.tensor_scalar(
                        out=ub[0:p_in, 2:2 + W], in0=tb[0:p_in],
                        scalar1=-2.0, scalar2=1.0, op0=Alu.mult, op1=Alu.add)

                    # s = p*u (in place over pb)
                    nc.vector.tensor_mul(out=pb[0:p_in], in0=pb[0:p_in],
                                         in1=ub[0:p_in, 2:2 + W])

                    # loss = ln(exp(s)*own + 1): own=0 zeroes overlap rows
                    g = g_pool.tile([128, W], bf16)
                    nc.scalar.activation(out=g[0:p_in], in_=pb[0:p_in],
                                         func=Act.Exp)
                    loss = loss_pool.tile([128, W], bf16)
                    nc.scalar.activation(out=loss[0:p_in], in_=g[0:p_in],
                                         func=Act.Ln, bias=1.0)

                    # v[j] = u[j] + u[j+1] (first half of the row 5-window)
                    vb = v_bufs[idx % 3]
                    nc.gpsimd.tensor_add(out=vb[0:p_in, 0:1026],
                                         in0=ub[0:p_in, 0:1026],
                                         in1=ub[0:p_in, 1:1027])

                    # Su = A @ (v(-2) + v(0) + u(+2)): 3 accumulating matmuls
                    sup = psum_pool.tile([128, W], f32)
                    for h in (0, 512):
                        nc.tensor.matmul(sup[:, h:h + 512], a_sb[0:p_in, :],
                                         vb[0:p_in, h:h + 512],
                                         start=True, stop=False)
                        nc.tensor.matmul(sup[:, h:h + 512], a_sb[0:p_in, :],
                                         vb[0:p_in, h + 2:h + 514],
                                         start=False, stop=False)
                        nc.tensor.matmul(sup[:, h:h + 512], a_sb[0:p_in, :],
                                         ub[0:p_in, h + 4:h + 516],
                                         start=False, stop=True)
                        # sum(loss), accumulated across all tiles
                        nc.tensor.matmul(lsum[:, h:h + 512],
                                         ones_sb[0:p_in, :],
                                         loss[0:p_in, h:h + 512],
                                         start=(idx == 0),
                                         stop=(idx == NTILES - 1))

                    # |Su| to SBUF (ScalarE Abs on the PSUM read)
                    asu = asu_pool.tile([128, W], bf16)
                    nc.scalar.activation(out=asu[:], in_=sup[:], func=Act.Abs)

                    # le = (|Su| < V-0.5) * loss, accumulated per partition.
                    # main covers interior cols; image-edge cols/rows redone
                    # with their own thresholds into separate accumulators.
                    scr = scr_pool.tile([128, W], bf16)
                    nc.vector.scalar_tensor_tensor(
                        out=scr[0:p_in, 2:W - 2], in0=asu[0:p_in, 2:W - 2],
                        scalar=thrm_sb[0:p_in, 0:1], in1=loss[0:p_in, 2:W - 2],
                        op0=Alu.is_lt, op1=Alu.mult,
                        accum_out=stats[0:p_in, idx:idx + 1])
                    # cols {0, W-1}: vw=3; cols {1, W-2}: vw=4 (strided pairs)
                    for coff, cstride, thr_sb, base in (
                            (0, W - 1, thra_sb, 40), (1, W - 3, thrb_sb, 80)):
                        asu_e = bass.AP(
                            tensor=asu[:].tensor,
                            offset=asu[:].offset + coff,
                            ap=[[asu[:].ap[0][0], p_in], [cstride, 2]])
                        loss_e = bass.AP(
                            tensor=loss[:].tensor,
                            offset=loss[:].offset + coff,
                            ap=[[loss[:].ap[0][0], p_in], [cstride, 2]])
                        scr_e = bass.AP(
                            tensor=scr[:].tensor,
                            offset=scr[:].offset + coff,
                            ap=[[scr[:].ap[0][0], p_in], [cstride, 2]])
                        nc.vector.scalar_tensor_tensor(
                            out=scr_e, in0=asu_e, scalar=thr_sb[0:p_in, 0:1],
                            in1=loss_e, op0=Alu.is_lt, op1=Alu.mult,
                            accum_out=stats[0:p_in, base + idx:base + idx + 1])
                    # image top/bottom rows (interior cols, vw=5)
                    if t == 0 or t == NT - 1:
                        rr = 2 if t == 0 else 32
                        nc.vector.scalar_tensor_tensor(
                            out=scr[0:rr, 2:W - 2], in0=asu[0:rr, 2:W - 2],
                            scalar=thrr_sb[0:rr, 0:1], in1=loss[0:rr, 2:W - 2],
                            op0=Alu.is_lt, op1=Alu.mult,
                            accum_out=stats[0:rr, 120 + rowidx:121 + rowidx])
                        rowidx = (rowidx + 1) % 8
                    idx += 1

            red = sing.tile([128, 2], f32, tag="red")
            nc.vector.memset(red[:, 0:1], 0.0)
            nc.vector.reduce_sum(out=red[:, 1:2], in_=stats[:, :],
                                 axis=mybir.AxisListType.X)
            nc.sync.dma_start(out=out_d[:], in_=red[:])
            lsum_sb = sing.tile([1, 1024], f32, tag="lsum_sb")
            nc.vector.tensor_copy(out=lsum_sb[:], in_=lsum[:])
            nc.sync.dma_start(out=out2_d[:], in_=lsum_sb[:])

    _split_multi_waits(nc, mybir)
    return nc


def _get_nc():
    if "nc" not in _CACHED:
        _CACHED["nc"] = _build_nc()
    return _CACHED["nc"]


def run(pred: np.ndarray, target: np.ndarray, trace: bool = False):
    """Returns (result_scalar, BassKernelResults)."""
    from concourse import bass_utils

    nc = _get_nc()
    statics = _statics()
    pred = np.ascontiguousarray(np.asarray(pred).reshape(B * H, W),
                                dtype=np.float32)
    target = np.ascontiguousarray(np.asarray(target).reshape(B * H, W),
                                  dtype=np.float32)
    in_maps = []
    for c in range(NCORES):
        m = dict(statics)
        m["pred"] = pred[c * ROWS:(c + 1) * ROWS]
        m["target"] = target[c * ROWS:(c + 1) * ROWS]
        in_maps.append(m)
    res = bass_utils.run_bass_kernel_spmd(
        nc, in_maps, core_ids=list(range(NCORES)), trace=trace)
    s_loss = 0.0
    s_le = 0.0
    for r in res.results:
        s_loss += r["out2"].astype(np.float64).sum()
        s_le += r["out"].astype(np.float64)[:, 1].sum()
    val = np.float32((s_loss - 0.9 * s_le) / N_TOT)
    return np.asarray(val, dtype=np.float32), res


def kernel(pred: np.ndarray, target: np.ndarray) -> np.ndarray:
    val, _ = run(pred, target, trace=False)
    return val


if __name__ == "__main__":
    rng = np.random.default_rng(0)
    p = rng.standard_normal((B, 1, H, W)).astype(np.float32)
    t = rng.integers(0, 2, (B, 1, H, W)).astype(np.float32)
    print(kernel(pred=p, target=t))
